# revision 2
# baseline (speedup 1.0000x reference)
"""Trainium2 Bass kernel for nn_Critic (MLP value function + GAE).

Sharding: batch B=2048 split across 8 NeuronCores (256 each). MLP params
replicated. The time recurrence (reverse GAE scan) is independent per batch
element, so no cross-core communication.

v2 strategy (vs v1's 3-pass bf16 hi/lo split + PE transposes):
  - Single-pass bf16 matmuls everywhere (fp32 PSUM accumulate). Measured
    numpy emulation gives rel err ~5e-3 vs the 2e-2 gate.
  - states are transposed to feature-major [D, T*B] bf16 on the HOST, so
    the kernel does zero PE transposes and zero hi/lo splits; the moving
    operand tiles DMA straight from HBM.
  - Column order is b-major with reversed time per batch segment:
    col = b*17 + r, r = 16-t. The MLP is row-independent so any column
    permutation works; this one makes the GAE a per-partition scan.
  - Work is streamed in chunks of N=512 columns (8x512 + 1x256): matmul
    free dim 512 = one PSUM bank, near-peak PE streaming.
  - value head: Wo is the stationary operand ([128,1] slices) so values
    land in PSUM [1, N]; rows are appended into value_row [1, 4352] and
    reshaped once via a SBUF->SBUF DMA into valP [128, 34] (2 batch
    segments of 17 per partition).
  - GAE: a handful of [128,16] VectorE ops + tensor_tensor_scan per
    segment half. disc/dl (elementwise scalings of `cont`) and all
    reversals/permutations are host-side input prep.
"""

import sys

sys.path.insert(0, "/opt/trn_rl_repo")

import numpy as np

T, B, D, H = 16, 2048, 2048, 1024
NCORES = 8
BC = B // NCORES  # 256 batch per core
TP1 = T + 1
TOT = TP1 * BC  # 4352 MLP rows per core
DISCOUNT, LAMBDA = 0.99, 0.95
P = 128
KD = D // P  # 16 k-tiles for layer 0
KH = H // P  # 8 k-tiles for layers 1,2,out
MH = H // P  # 8 m-tiles of hidden units
CHUNKS = [512] * 8 + [256]  # sum = 4352
SEG = TP1  # 17 values per batch segment
NSEG_P = 2  # batch segments per partition in valP (128*2 = 256 = BC)

_NC_CACHE = None


def _build():
    import concourse.bacc as bacc
    import concourse.mybir as mybir
    from concourse.tile import TileContext

    F32 = mybir.dt.float32
    BF16 = mybir.dt.bfloat16
    ALU = mybir.AluOpType
    ACTF = mybir.ActivationFunctionType

    nc = bacc.Bacc(None, target_bir_lowering=False, debug=False)

    statesT_h = nc.declare_dram_parameter("statesT", [D, TOT], BF16, isOutput=False)
    w0_h = nc.declare_dram_parameter("W0", [D, H], BF16, isOutput=False)
    b0_h = nc.declare_dram_parameter("b0", [H, 1], F32, isOutput=False)
    w1_h = nc.declare_dram_parameter("W1", [H, H], BF16, isOutput=False)
    b1_h = nc.declare_dram_parameter("b1", [H, 1], F32, isOutput=False)
    w2_h = nc.declare_dram_parameter("W2", [H, H], BF16, isOutput=False)
    b2_h = nc.declare_dram_parameter("b2", [H, 1], F32, isOutput=False)
    wo_h = nc.declare_dram_parameter("Wo", [H, 1], BF16, isOutput=False)
    bo_h = nc.declare_dram_parameter("bo", [1, 1], F32, isOutput=False)
    rew_h = nc.declare_dram_parameter("rewP", [P, 2 * T], F32, isOutput=False)
    disc_h = nc.declare_dram_parameter("discP", [P, 2 * T], F32, isOutput=False)
    dl_h = nc.declare_dram_parameter("dlP", [P, 2 * T], F32, isOutput=False)
    ret_h = nc.declare_dram_parameter("retP", [P, 2 * T], F32, isOutput=True)
    val_h = nc.declare_dram_parameter("valP", [P, 2 * T], F32, isOutput=True)

    with TileContext(nc) as tc:
        with (
            tc.tile_pool(name="wpool", bufs=1) as wpool,
            tc.tile_pool(name="stpool", bufs=1) as stpool,
            tc.tile_pool(name="hpool", bufs=1) as hpool,
            tc.tile_pool(name="tmp", bufs=3) as tmppool,
            tc.tile_pool(name="gae", bufs=1) as gaepool,
            tc.tile_pool(name="psA", bufs=4, space="PSUM") as psApool,
            tc.tile_pool(name="psV", bufs=2, space="PSUM") as psVpool,
        ):
            # ---- persistent weights / constants ----
            # W0 + states go on the sync HWDGE queue; W1/W2/rest on the
            # scalar queue so the two streams don't serialize.
            def load_weight(dram_h, name, nk, eng):
                tiles = []
                for k in range(nk):
                    wt = wpool.tile([P, H], BF16, name=f"{name}{k}", tag=f"{name}{k}")
                    eng.dma_start(out=wt[:], in_=dram_h[k * P : (k + 1) * P, :])
                    tiles.append(wt)
                return tiles

            w0 = load_weight(w0_h, "w0", KD, nc.sync)
            w1 = load_weight(w1_h, "w1", KH, nc.scalar)
            w2 = load_weight(w2_h, "w2", KH, nc.scalar)
            wosb = wpool.tile([P, KH], BF16, name="wosb", tag="wosb")
            for k in range(KH):
                nc.scalar.dma_start(
                    out=wosb[:, k : k + 1], in_=wo_h[k * P : (k + 1) * P, :]
                )
            bsb = []
            for li, bh in enumerate((b0_h, b1_h, b2_h)):
                bt = wpool.tile([P, MH], F32, name=f"bsb{li}", tag=f"bsb{li}")
                for m in range(MH):
                    nc.scalar.dma_start(out=bt[:, m : m + 1], in_=bh[m * P : (m + 1) * P, :])
                bsb.append(bt)
            bosb = wpool.tile([1, 1], F32, name="bosb", tag="bosb")
            nc.scalar.dma_start(out=bosb[:], in_=bo_h[:])

            rewsb = gaepool.tile([P, 2 * T], F32, name="rewsb", tag="rewsb")
            nc.scalar.dma_start(out=rewsb[:], in_=rew_h[:])
            discsb = gaepool.tile([P, 2 * T], F32, name="discsb", tag="discsb")
            nc.scalar.dma_start(out=discsb[:], in_=disc_h[:])
            dlsb = gaepool.tile([P, 2 * T], F32, name="dlsb", tag="dlsb")
            nc.scalar.dma_start(out=dlsb[:], in_=dl_h[:])

            value_row = gaepool.tile([1, TOT], F32, name="value_row", tag="value_row")

            # ---- streamed MLP over column chunks ----
            c0 = 0
            for n in CHUNKS:
                sts = []
                for k in range(KD):
                    st = stpool.tile([P, n], BF16, name=f"st{k}", tag="st", bufs=2 * KD)
                    nc.sync.dma_start(
                        out=st[:], in_=statesT_h[k * P : (k + 1) * P, c0 : c0 + n]
                    )
                    sts.append(st)

                hs = []
                for li, (wts, bias, nk) in enumerate(
                    ((w0, bsb[0], KD), (w1, bsb[1], KH), (w2, bsb[2], KH))
                ):
                    hout = hpool.tile([P, MH * n], BF16, name=f"h{li}", tag=f"h{li}", bufs=2)
                    for m in range(MH):
                        ms = slice(m * P, (m + 1) * P)
                        psm = psApool.tile([P, n], F32, name="psm", tag="psm")
                        for k in range(nk):
                            rhs = sts[k][:] if li == 0 else hs[-1][:, k * n : (k + 1) * n]
                            nc.tensor.matmul(
                                psm[:],
                                lhsT=wts[k][:, ms],
                                rhs=rhs,
                                start=(k == 0),
                                stop=(k == nk - 1),
                                skip_group_check=True,
                            )
                        # ELU(z+b) = min(exp(z+b)-1, relu(z+b))
                        e = tmppool.tile([P, n], F32, name="e", tag="e")
                        nc.scalar.activation(e[:], psm[:], ACTF.Exp, bias=bias[:, m : m + 1])
                        rl = tmppool.tile([P, n], F32, name="rl", tag="rl")
                        nc.vector.tensor_scalar(
                            rl[:], psm[:], bias[:, m : m + 1], 0.0, ALU.add, ALU.max
                        )
                        nc.vector.scalar_tensor_tensor(
                            hout[:, m * n : (m + 1) * n],
                            e[:],
                            1.0,
                            rl[:],
                            ALU.subtract,
                            ALU.min,
                        )
                    hs.append(hout)

                # value head: Wo stationary [128,1] -> value lands [1, n]
                pv = psVpool.tile([1, n], F32, name="pv", tag="pv")
                for k in range(KH):
                    nc.tensor.matmul(
                        pv[:],
                        lhsT=wosb[:, k : k + 1],
                        rhs=hs[2][:, k * n : (k + 1) * n],
                        start=(k == 0),
                        stop=(k == KH - 1),
                        skip_group_check=True,
                    )
                nc.vector.tensor_scalar_add(
                    value_row[0:1, c0 : c0 + n], pv[:], bosb[0:1, 0:1]
                )
                c0 += n

            # ---- GAE ----
            # value_row [1, 4352] -> valP [128, 34]: partition p holds batch
            # segments b=2p,2p+1; within a segment index r: value(t=16-r).
            valP = gaepool.tile([P, NSEG_P * SEG], F32, name="valPsb", tag="valPsb")
            nc.sync.dma_start(out=valP[:], in_=value_row[:])

            dtt = gaepool.tile([P, 2 * T], F32, name="dtt", tag="dtt")
            adv = gaepool.tile([P, 2 * T], F32, name="adv", tag="adv")
            retP = gaepool.tile([P, 2 * T], F32, name="retP", tag="retP")
            for s in range(NSEG_P):
                ss = slice(s * T, (s + 1) * T)
                vnext = valP[:, s * SEG : s * SEG + T]
                vcur = valP[:, s * SEG + 1 : s * SEG + 1 + T]
                nc.vector.tensor_mul(dtt[:, ss], discsb[:, ss], vnext)
                nc.vector.tensor_add(dtt[:, ss], dtt[:, ss], rewsb[:, ss])
                nc.vector.tensor_sub(dtt[:, ss], dtt[:, ss], vcur)
                nc.vector.tensor_tensor_scan(
                    adv[:, ss], dlsb[:, ss], dtt[:, ss], 0.0, ALU.mult, ALU.add
                )
                nc.vector.tensor_add(retP[:, ss], adv[:, ss], vcur)
                nc.sync.dma_start(out=val_h[:, ss], in_=vcur)
            nc.sync.dma_start(out=ret_h[:], in_=retP[:])

    nc.compile()
    return nc


def _get_nc():
    global _NC_CACHE
    if _NC_CACHE is None:
        _NC_CACHE = _build()
    return _NC_CACHE


def _make_in_maps(inputs):
    import ml_dtypes

    BF = ml_dtypes.bfloat16
    states = np.asarray(inputs["states"], dtype=np.float32)
    reward = np.asarray(inputs["reward"], dtype=np.float32)
    cont = np.asarray(inputs["cont"], dtype=np.float32)

    # Feature-major states, b-major columns with reversed time:
    # full[d, b, r] = states[16-r, b, d] in bf16.
    st_bf = states.astype(BF)
    full = np.ascontiguousarray(st_bf[::-1].transpose(2, 1, 0))  # [D, B, TP1]

    def wbf(name):
        return np.ascontiguousarray(np.asarray(inputs[name], np.float32).astype(BF))

    W0, W1, W2 = wbf("W0"), wbf("W1"), wbf("W2")
    Wo = np.ascontiguousarray(
        np.asarray(inputs["Wo"], np.float32).astype(BF).reshape(H, 1)
    )
    b0 = np.ascontiguousarray(np.asarray(inputs["b0"], np.float32).reshape(H, 1))
    b1 = np.ascontiguousarray(np.asarray(inputs["b1"], np.float32).reshape(H, 1))
    b2 = np.ascontiguousarray(np.asarray(inputs["b2"], np.float32).reshape(H, 1))
    bo = np.ascontiguousarray(np.asarray(inputs["bo"], np.float32).reshape(1, 1))

    in_maps = []
    for c in range(NCORES):
        sl = slice(c * BC, (c + 1) * BC)
        # rewP[p, s*16+j] = reward[15-j, 2p+s]; disc uses cont[16-j].
        rr = reward[::-1, sl]  # [T, BC], rr[j] = reward[15-j]
        cc = cont[1:][::-1, sl]  # [T, BC], cc[j] = cont[16-j]
        rewP = np.ascontiguousarray(rr.T.reshape(P, 2 * T))
        discP = np.ascontiguousarray((DISCOUNT * cc).T.reshape(P, 2 * T))
        dlP = np.ascontiguousarray((DISCOUNT * LAMBDA * cc).T.reshape(P, 2 * T))
        in_maps.append(
            {
                "statesT": np.ascontiguousarray(full[:, sl, :]).reshape(D, TOT),
                "W0": W0,
                "b0": b0,
                "W1": W1,
                "b1": b1,
                "W2": W2,
                "b2": b2,
                "Wo": Wo,
                "bo": bo,
                "rewP": rewP,
                "discP": discP,
                "dlP": dlP,
            }
        )
    return in_maps


def _run(inputs, trace=False):
    from concourse.bass_utils import run_bass_kernel_spmd

    nc = _get_nc()
    in_maps = _make_in_maps(inputs)
    bkr = run_bass_kernel_spmd(nc, in_maps, list(range(NCORES)), trace=trace)
    ret = np.empty((T, B), np.float32)
    val = np.empty((T, B), np.float32)
    for c in range(NCORES):
        sl = slice(c * BC, (c + 1) * BC)
        # retP[p, s*16+j] -> ret[15-j, 2p+s]
        rp = bkr.results[c]["retP"].reshape(P, 2, T)[:, :, ::-1]  # [p, s, t]
        vp = bkr.results[c]["valP"].reshape(P, 2, T)[:, :, ::-1]
        ret[:, sl] = rp.transpose(2, 0, 1).reshape(T, BC)
        val[:, sl] = vp.transpose(2, 0, 1).reshape(T, BC)
    return (ret, val), bkr


def kernel(**inputs):
    out, _ = _run(inputs, trace=False)
    return out


# revision 3
# speedup vs baseline: 1.0632x; 1.0632x over previous
"""Trainium2 Bass kernel for nn_Critic (MLP value function + GAE).

Sharding: batch B=2048 split across 8 NeuronCores (256 each). MLP params
replicated. The time recurrence (reverse GAE scan) is independent per batch
element, so no cross-core communication.

v3 strategy:
  - Single-pass bf16 matmuls everywhere (fp32 PSUM accumulate). Measured
    numpy emulation gives rel err ~5e-3 vs the 2e-2 gate.
  - states are transposed to feature-major bf16 on the HOST, so the kernel
    does zero PE transposes and zero hi/lo splits.
  - Column order is b-major with reversed time per batch segment:
    col = b*17 + r, r = 16-t. The MLP is row-independent so any column
    permutation works; this one makes the GAE a per-partition scan.
  - Work is streamed in chunks of N=512 columns (8x512 + 1x256): matmul
    free dim 512 = one PSUM bank, near-peak PE streaming.
  - All inputs are host-packed p-major so every SBUF tile loads with ONE
    large DMA (the v2 trace showed 164 small DMAs cost ~600ns of issue
    time each and stalled the PE for ~35us at start). The first chunk's
    states + W0 are split into 1MB pieces across both HWDGE queues so the
    PE can start after ~3us.
  - A few warm-up matmuls on zeroed tiles run during the initial DMA wait
    so the PE HAM clock-gate is at 2.4GHz when real work lands (v2 paid
    ~21us of cold-clock matmuls).
  - value head: Wo is the stationary operand ([128,1] slices) so values
    land in PSUM [1, N]; rows are appended into value_row [1, 4352] and
    reshaped once via a SBUF->SBUF DMA into valP [128, 34] (2 batch
    segments of 17 per partition).
  - GAE: a handful of [128,16] VectorE ops + tensor_tensor_scan per
    segment half. disc/dl (elementwise scalings of `cont`) and all
    reversals/permutations are host-side input prep.
"""

import sys

sys.path.insert(0, "/opt/trn_rl_repo")

import numpy as np

T, B, D, H = 16, 2048, 2048, 1024
NCORES = 8
BC = B // NCORES  # 256 batch per core
TP1 = T + 1
TOT = TP1 * BC  # 4352 MLP rows per core
DISCOUNT, LAMBDA = 0.99, 0.95
P = 128
KD = D // P  # 16 k-tiles for layer 0
KH = H // P  # 8 k-tiles for layers 1,2,out
MH = H // P  # 8 m-tiles of hidden units
CHUNKS = [512] * 8 + [256]  # sum = 4352
SEG = TP1  # 17 values per batch segment
NWARM = 8  # warm-up matmuls

_NC_CACHE = None


def _build():
    import concourse.bacc as bacc
    import concourse.mybir as mybir
    from concourse.tile import TileContext

    F32 = mybir.dt.float32
    BF16 = mybir.dt.bfloat16
    ALU = mybir.AluOpType
    ACTF = mybir.ActivationFunctionType

    nc = bacc.Bacc(None, target_bir_lowering=False, debug=False)

    # statesT: per chunk j (n cols), p-major halves: [2, 128, 8, n];
    # last chunk (n=256) is one [128, 16, n] block. Flattened to rows of 1024.
    statesT_h = nc.declare_dram_parameter("statesT", [D * TOT // 1024, 1024], BF16, isOutput=False)
    # W0: [4 parts, 128, 4, 1024] part/p-major; W1/W2: [128, 8, 1024] p-major.
    w0_h = nc.declare_dram_parameter("W0t", [D, H], BF16, isOutput=False)
    w1_h = nc.declare_dram_parameter("W1t", [H, H], BF16, isOutput=False)
    w2_h = nc.declare_dram_parameter("W2t", [H, H], BF16, isOutput=False)
    wo_h = nc.declare_dram_parameter("WoP", [P, KH], BF16, isOutput=False)
    bias_h = nc.declare_dram_parameter("biasP", [P, 3 * MH], F32, isOutput=False)
    bo_h = nc.declare_dram_parameter("bo", [1, 1], F32, isOutput=False)
    gae_h = nc.declare_dram_parameter("gaeP", [P, 6 * T], F32, isOutput=False)
    ret_h = nc.declare_dram_parameter("retP", [P, 2 * T], F32, isOutput=True)
    val_h = nc.declare_dram_parameter("valP", [P, 2 * T], F32, isOutput=True)

    with TileContext(nc) as tc:
        with (
            tc.tile_pool(name="wpool", bufs=1) as wpool,
            tc.tile_pool(name="stpool", bufs=1) as stpool,
            tc.tile_pool(name="hpool", bufs=1) as hpool,
            tc.tile_pool(name="tmp", bufs=3) as tmppool,
            tc.tile_pool(name="gae", bufs=1) as gaepool,
            tc.tile_pool(name="psA", bufs=4, space="PSUM") as psApool,
            tc.tile_pool(name="psV", bufs=2, space="PSUM") as psVpool,
            tc.tile_pool(name="psW", bufs=1, space="PSUM") as psWpool,
        ):
            # ---- PE warm-up on zeroed tiles (overlaps the first DMAs) ----
            zw = wpool.tile([P, P], BF16, name="zw", tag="zw")
            nc.vector.memset(zw[:], 0.0)
            zx = wpool.tile([P, 512], BF16, name="zx", tag="zx")
            nc.vector.memset(zx[:], 0.0)
            zp = psWpool.tile([P, 512], F32, name="zp", tag="zp")
            for _ in range(NWARM):
                nc.tensor.matmul(
                    zp[:], lhsT=zw[:], rhs=zx[:], start=True, stop=True,
                    skip_group_check=True,
                )

            # ---- weights / constants (one big DMA per tensor) ----
            w0all = wpool.tile([P, KD * H], BF16, name="w0all", tag="w0all")
            for q in range(4):
                nc.scalar.dma_start(
                    out=w0all[:, q * 4 * H : (q + 1) * 4 * H],
                    in_=w0_h[q * 512 : (q + 1) * 512, :],
                )
            w1all = wpool.tile([P, KH * H], BF16, name="w1all", tag="w1all")
            nc.scalar.dma_start(out=w1all[:], in_=w1_h[:])
            w2all = wpool.tile([P, KH * H], BF16, name="w2all", tag="w2all")
            nc.scalar.dma_start(out=w2all[:], in_=w2_h[:])
            wall = (w0all, w1all, w2all)

            wosb = wpool.tile([P, KH], BF16, name="wosb", tag="wosb")
            nc.sync.dma_start(out=wosb[:], in_=wo_h[:])
            biasP = wpool.tile([P, 3 * MH], F32, name="biasP", tag="biasP")
            nc.sync.dma_start(out=biasP[:], in_=bias_h[:])
            bosb = wpool.tile([1, 1], F32, name="bosb", tag="bosb")
            nc.sync.dma_start(out=bosb[:], in_=bo_h[:])
            gaesb = gaepool.tile([P, 6 * T], F32, name="gaesb", tag="gaesb")
            nc.sync.dma_start(out=gaesb[:], in_=gae_h[:])
            rewsb = gaesb[:, 0 : 2 * T]
            discsb = gaesb[:, 2 * T : 4 * T]
            dlsb = gaesb[:, 4 * T : 6 * T]

            value_row = gaepool.tile([1, TOT], F32, name="value_row", tag="value_row")

            # ---- streamed MLP over column chunks ----
            c0 = 0
            row0 = 0
            for n in CHUNKS:
                st_all = stpool.tile([P, KD * n], BF16, name="st", tag="st", bufs=2)
                nrows = KD * P * n // 1024  # 1024 (n=512) or 512 (n=256)
                ndma = 2 if n == 512 else 1
                for h in range(ndma):
                    hr = nrows // ndma
                    nc.sync.dma_start(
                        out=st_all[:, h * (KD * n // ndma) : (h + 1) * (KD * n // ndma)],
                        in_=statesT_h[row0 + h * hr : row0 + (h + 1) * hr, :],
                    )
                row0 += nrows

                hs = []
                for li, nk in ((0, KD), (1, KH), (2, KH)):
                    rhs_src = st_all if li == 0 else hs[-1]
                    hout = hpool.tile([P, MH * n], BF16, name=f"h{li}", tag=f"h{li}", bufs=2)
                    for m in range(MH):
                        psm = psApool.tile([P, n], F32, name="psm", tag="psm")
                        for k in range(nk):
                            nc.tensor.matmul(
                                psm[:],
                                lhsT=wall[li][:, k * H + m * P : k * H + (m + 1) * P],
                                rhs=rhs_src[:, k * n : (k + 1) * n],
                                start=(k == 0),
                                stop=(k == nk - 1),
                                skip_group_check=True,
                            )
                        # ELU(z+b) = min(exp(z+b)-1, relu(z+b))
                        bcol = biasP[:, li * MH + m : li * MH + m + 1]
                        e = tmppool.tile([P, n], F32, name="e", tag="e")
                        nc.scalar.activation(e[:], psm[:], ACTF.Exp, bias=bcol)
                        rl = tmppool.tile([P, n], F32, name="rl", tag="rl")
                        nc.vector.tensor_scalar(
                            rl[:], psm[:], bcol, 0.0, ALU.add, ALU.max
                        )
                        nc.vector.scalar_tensor_tensor(
                            hout[:, m * n : (m + 1) * n],
                            e[:],
                            1.0,
                            rl[:],
                            ALU.subtract,
                            ALU.min,
                        )
                    hs.append(hout)

                # value head: Wo stationary [128,1] -> value lands [1, n]
                pv = psVpool.tile([1, n], F32, name="pv", tag="pv")
                for k in range(KH):
                    nc.tensor.matmul(
                        pv[:],
                        lhsT=wosb[:, k : k + 1],
                        rhs=hs[2][:, k * n : (k + 1) * n],
                        start=(k == 0),
                        stop=(k == KH - 1),
                        skip_group_check=True,
                    )
                nc.vector.tensor_scalar_add(
                    value_row[0:1, c0 : c0 + n], pv[:], bosb[0:1, 0:1]
                )
                c0 += n

            # ---- GAE ----
            # value_row [1, 4352] -> valP [128, 34]: partition p holds batch
            # segments b=2p,2p+1; within a segment index r: value(t=16-r).
            valP = gaepool.tile([P, 2 * SEG], F32, name="valPsb", tag="valPsb")
            nc.sync.dma_start(out=valP[:], in_=value_row[:])

            dtt = gaepool.tile([P, 2 * T], F32, name="dtt", tag="dtt")
            adv = gaepool.tile([P, 2 * T], F32, name="adv", tag="adv")
            retP = gaepool.tile([P, 2 * T], F32, name="retP", tag="retP")
            for s in range(2):
                ss = slice(s * T, (s + 1) * T)
                vnext = valP[:, s * SEG : s * SEG + T]
                vcur = valP[:, s * SEG + 1 : s * SEG + 1 + T]
                nc.vector.tensor_mul(dtt[:, ss], discsb[:, ss], vnext)
                nc.vector.tensor_add(dtt[:, ss], dtt[:, ss], rewsb[:, ss])
                nc.vector.tensor_sub(dtt[:, ss], dtt[:, ss], vcur)
                nc.vector.tensor_tensor_scan(
                    adv[:, ss], dlsb[:, ss], dtt[:, ss], 0.0, ALU.mult, ALU.add
                )
                nc.vector.tensor_add(retP[:, ss], adv[:, ss], vcur)
                nc.sync.dma_start(out=val_h[:, ss], in_=vcur)
            nc.sync.dma_start(out=ret_h[:], in_=retP[:])

    nc.compile()
    return nc


def _get_nc():
    global _NC_CACHE
    if _NC_CACHE is None:
        _NC_CACHE = _build()
    return _NC_CACHE


def _pack_pmajor(w, nk):
    # [nk*128, cols] -> p-major [128, nk, cols] flattened back to same shape
    cols = w.shape[1]
    return np.ascontiguousarray(
        w.reshape(nk, P, cols).transpose(1, 0, 2).reshape(nk * P, cols)
    )


def _make_in_maps(inputs):
    import ml_dtypes

    BF = ml_dtypes.bfloat16
    states = np.asarray(inputs["states"], dtype=np.float32)
    reward = np.asarray(inputs["reward"], dtype=np.float32)
    cont = np.asarray(inputs["cont"], dtype=np.float32)

    # Feature-major states, b-major columns with reversed time:
    # full[d, b, r] = states[16-r, b, d] in bf16.
    st_bf = states.astype(BF)
    full = np.ascontiguousarray(st_bf[::-1].transpose(2, 1, 0))  # [D, B, TP1]

    W0 = np.asarray(inputs["W0"], np.float32).astype(BF)
    W1 = np.asarray(inputs["W1"], np.float32).astype(BF)
    W2 = np.asarray(inputs["W2"], np.float32).astype(BF)
    # W0: [4 parts, 128, 4, 1024] part-major then p-major
    W0t = np.ascontiguousarray(
        W0.reshape(4, 4, P, H).transpose(0, 2, 1, 3).reshape(D, H)
    )
    W1t = _pack_pmajor(W1, KH)
    W2t = _pack_pmajor(W2, KH)
    WoP = np.ascontiguousarray(
        np.asarray(inputs["Wo"], np.float32).astype(BF).reshape(KH, P).T
    )
    b3 = np.stack(
        [np.asarray(inputs[k], np.float32) for k in ("b0", "b1", "b2")]
    )  # [3, 1024]
    biasP = np.ascontiguousarray(b3.reshape(3, MH, P).transpose(2, 0, 1).reshape(P, 3 * MH))
    bo = np.ascontiguousarray(np.asarray(inputs["bo"], np.float32).reshape(1, 1))

    in_maps = []
    for c in range(NCORES):
        sl = slice(c * BC, (c + 1) * BC)
        # statesT for this core: [D, 4352] b-major/rev-t columns, then
        # per chunk: halves x [128, 8|16, n] p-major, flattened.
        stT = full[:, sl, :].reshape(D, TOT)
        blocks = []
        c0 = 0
        for n in CHUNKS:
            blk = stT[:, c0 : c0 + n].reshape(KD, P, n)  # [k, p, n]
            ndma = 2 if n == 512 else 1
            kk = KD // ndma
            blocks.append(
                np.ascontiguousarray(
                    blk.reshape(ndma, kk, P, n).transpose(0, 2, 1, 3)
                ).reshape(-1)
            )
            c0 += n
        statesT = np.concatenate(blocks).reshape(D * TOT // 1024, 1024)

        # rewP[p, s*16+j] = reward[15-j, 2p+s]; disc uses cont[16-j].
        rr = reward[::-1, sl]  # [T, BC], rr[j] = reward[15-j]
        cc = cont[1:][::-1, sl]  # [T, BC], cc[j] = cont[16-j]
        rewP = rr.T.reshape(P, 2 * T)
        discP = (DISCOUNT * cc).T.reshape(P, 2 * T)
        dlP = (DISCOUNT * LAMBDA * cc).T.reshape(P, 2 * T)
        gaeP = np.ascontiguousarray(np.concatenate([rewP, discP, dlP], axis=1))
        in_maps.append(
            {
                "statesT": statesT,
                "W0t": W0t,
                "W1t": W1t,
                "W2t": W2t,
                "WoP": WoP,
                "biasP": biasP,
                "bo": bo,
                "gaeP": gaeP,
            }
        )
    return in_maps


def _run(inputs, trace=False):
    from concourse.bass_utils import run_bass_kernel_spmd

    nc = _get_nc()
    in_maps = _make_in_maps(inputs)
    bkr = run_bass_kernel_spmd(nc, in_maps, list(range(NCORES)), trace=trace)
    ret = np.empty((T, B), np.float32)
    val = np.empty((T, B), np.float32)
    for c in range(NCORES):
        sl = slice(c * BC, (c + 1) * BC)
        # retP[p, s*16+j] -> ret[15-j, 2p+s]
        rp = bkr.results[c]["retP"].reshape(P, 2, T)[:, :, ::-1]  # [p, s, t]
        vp = bkr.results[c]["valP"].reshape(P, 2, T)[:, :, ::-1]
        ret[:, sl] = rp.transpose(2, 0, 1).reshape(T, BC)
        val[:, sl] = vp.transpose(2, 0, 1).reshape(T, BC)
    return (ret, val), bkr


def kernel(**inputs):
    out, _ = _run(inputs, trace=False)
    return out
